# revision 7
# baseline (speedup 1.0000x reference)
"""Bass/Trainium2 kernel for nn_BigramLM (dense transformer, 8 NeuronCores).

Sharding v2: token-split pairs. Cores (2b, 2b+1) split batch b's tokens at
128-tile granularity: parity 0 owns global token tiles {0,2,4,6}, parity 1
owns {1,3,5,7} (512 tokens each), balancing causal attention work. All
per-token ops (LN, QKV, FFN, residual) run on local tokens only — the
transformer body is computed exactly once across the pair. Per layer the
pair exchanges K/V through pair-shared HBM (addr_space='Shared'): each core
scatter-writes its K/V half, a tiny AllReduce acts as the cross-core
barrier, then both read the full-T K/V. Attention computes each core's 512
q-tokens against a uniform k-tile schedule with per-core mask data (SPMD-
identical instruction streams). The final vocab projection splits by vocab
half after one more xn exchange. All big matmuls run in bf16 (fp32 PSUM).

Storage token order (shared buffers & logits rows): global tiles
[0,2,4,6,1,3,5,7].
"""

import os
import sys

sys.path.insert(0, "/opt/trn_rl_repo")

import numpy as np

SKIP_CC = os.environ.get("SKIP_CC", "0") == "1"   # timing A/B only: no barriers

import concourse.bass as bass
import concourse.mybir as mybir
import concourse.tile as tile
from concourse import bacc
from concourse.bass_utils import run_bass_kernel_spmd
from concourse.masks import make_identity
from concourse.tile import add_dep_helper

F32 = mybir.dt.float32
BF16 = mybir.dt.bfloat16
I32 = mybir.dt.int32
AF = mybir.ActivationFunctionType
ALU = mybir.AluOpType
NP_BF16 = mybir.dt.np(BF16)

V, D, H, KD, B, T = 32000, 1024, 16, 64, 4, 1024
F = 4 * D
LAYERS = 4
P = 128
TL = 512               # local tokens per core
NJ = 4                 # local 128-token tiles
ND = D // P            # 8
NF = F // P            # 32
NPAIR = H // 2         # 8 head pairs
VSH = V // 2           # 16000 vocab cols per core
VC = 500
NVC = VSH // VC        # 32
EPS = 1e-5
SCALE = 1.0 / float(np.sqrt(KD))
RG = [[0, 1], [2, 3], [4, 5], [6, 7]]
STORDER = [0, 2, 4, 6, 1, 3, 5, 7]   # storage tile -> global tile


def _dram_ap(handle, offset, pattern):
    t = getattr(handle, "tensor", handle)
    offset = offset + getattr(handle, "offset", 0)
    return bass.AP(tensor=t, offset=offset, ap=[list(p) for p in pattern])


def build_program(repeat=1):
    nc = bacc.Bacc("TRN2", target_bir_lowering=False, debug=False, num_devices=8)

    tn = {}
    tn["x_idx"] = nc.dram_tensor("x_idx", [TL, 1], I32, kind="ExternalInput")
    tn["tok_emb"] = nc.dram_tensor("tok_emb", [V, D], F32, kind="ExternalInput")
    tn["pos"] = nc.dram_tensor("pos", [TL, D], F32, kind="ExternalInput")
    for nm, shp, dt in (
        ("wq", [D, H * KD], BF16), ("wk", [D, H * KD], BF16),
        ("wv", [D, H * KD], BF16),
        ("bq", [H * KD], F32), ("bk", [H * KD], F32), ("bv", [1, H * KD], F32),
        ("wo_aug", [KD + 1, H, KD + 1], BF16),
        ("w1", [D, F], BF16), ("b1", [F], F32),
        ("w2", [F, D], BF16), ("b2", [1, D], F32),
        ("ln1_g", [D], F32), ("ln1_b", [D], F32),
        ("ln2_g", [D], F32), ("ln2_b", [D], F32),
        ("lnf_g", [D], F32), ("lnf_b", [D], F32),
        ("wout", [D, VSH], BF16), ("bout", [1, VSH], F32),
        ("amask", [2, P, 512], BF16),
        ("kscat", [NPAIR, P], I32), ("vscat", [NJ, P], I32),
        ("xgat", [ND, P], I32),
    ):
        tn[nm] = nc.dram_tensor(nm, shp, dt, kind="ExternalInput")
    tn["logits_own"] = nc.dram_tensor("logits_own", [TL, VSH], BF16,
                                      kind="ExternalOutput")
    tn["logits_oth"] = nc.dram_tensor("logits_oth", [TL, VSH], BF16,
                                      kind="ExternalOutput")
    # pair-shared HBM scratch (double-buffered across layers for WAR safety)
    tn["sh_k0"] = nc.dram_tensor("sh_k0", [NPAIR * P * 2, TL], BF16,
                                 kind="Internal", addr_space="Shared")
    tn["sh_k1"] = nc.dram_tensor("sh_k1", [NPAIR * P * 2, TL], BF16,
                                 kind="Internal", addr_space="Shared")
    tn["sh_v0"] = nc.dram_tensor("sh_v0", [8 * P, H * KD], BF16,
                                 kind="Internal", addr_space="Shared")
    tn["sh_v1"] = nc.dram_tensor("sh_v1", [8 * P, H * KD], BF16,
                                 kind="Internal", addr_space="Shared")
    tn["sh_x"] = nc.dram_tensor("sh_x", [ND * P * 2, TL], BF16,
                                 kind="Internal", addr_space="Shared")

    with tile.TileContext(nc) as tc:
        prev_ccf = None
        for r in range(repeat):
            prev_ccf = _body(nc, tc, tn, sfx=f"_{r}" if r else "",
                             dep_from=prev_ccf)
    nc.compile()
    return nc


def _body(nc, tc, tn, sfx="", dep_from=None):
    const = tc.alloc_tile_pool(name="const" + sfx, bufs=1)
    pers = tc.alloc_tile_pool(name="pers" + sfx, bufs=1)
    small = tc.alloc_tile_pool(name="small" + sfx, bufs=3)
    ev = tc.alloc_tile_pool(name="ev" + sfx, bufs=2)
    fl_dram = tc.alloc_tile_pool(name="fl_dram" + sfx, bufs=2, space="DRAM")
    ps_big = tc.alloc_tile_pool(name="ps_big" + sfx, bufs=4, space="PSUM")
    ps_s = tc.alloc_tile_pool(name="ps_s" + sfx, bufs=2, space="PSUM")
    ps_y = tc.alloc_tile_pool(name="ps_y" + sfx, bufs=2, space="PSUM")
    static_pools = [const, pers, small, ev, fl_dram, ps_big, ps_s, ps_y]

    # ---------------- constants ----------------
    ident = const.tile([P, P], F32, tag="ident")
    make_identity(nc, ident)
    ident_bf = const.tile([P, P], BF16, tag="ident_bf")
    make_identity(nc, ident_bf)
    eps_t = const.tile([P, 1], F32, tag="eps_t")
    nc.vector.memset(eps_t, EPS)
    zero_sb = const.tile([1, P], F32, tag="zero_sb")
    nc.vector.memset(zero_sb, 0.0)
    ones_col = const.tile([P, 8, 2, 1], BF16, tag="ones_col")
    nc.vector.memset(ones_col, 1.0)

    bq_sb = const.tile([P, NPAIR], F32, tag="bq_sb")
    bk_sb = const.tile([P, NPAIR], F32, tag="bk_sb")
    b1_sb = const.tile([P, NF], F32, tag="b1_sb")
    nc.sync.dma_start(out=bq_sb, in_=_dram_ap(tn["bq"], 0, [[1, P], [P, NPAIR]]))
    nc.sync.dma_start(out=bk_sb, in_=_dram_ap(tn["bk"], 0, [[1, P], [P, NPAIR]]))
    nc.sync.dma_start(out=b1_sb, in_=_dram_ap(tn["b1"], 0, [[1, P], [P, NF]]))
    bv_bc = const.tile([P, H * KD], F32, tag="bv_bc")
    nc.sync.dma_start(out=bv_bc, in_=_dram_ap(tn["bv"], 0, [[0, P], [1, H * KD]]))
    b2_bc = const.tile([P, D], F32, tag="b2_bc")
    nc.sync.dma_start(out=b2_bc, in_=_dram_ap(tn["b2"], 0, [[0, P], [1, D]]))
    wo_sb = const.tile([KD + 1, H, KD + 1], BF16, tag="wo_sb")
    nc.sync.dma_start(out=wo_sb, in_=tn["wo_aug"][:, :, :])
    amask_sb = const.tile([P, 2, 512], BF16, tag="amask_sb")
    nc.sync.dma_start(out=amask_sb,
                      in_=_dram_ap(tn["amask"], 0, [[512, P], [P * 512, 2], [1, 512]]))
    kscat_sb = const.tile([P, NPAIR], I32, tag="kscat_sb")
    nc.sync.dma_start(out=kscat_sb, in_=_dram_ap(tn["kscat"], 0, [[1, P], [P, NPAIR]]))
    vscat_sb = const.tile([P, NJ], I32, tag="vscat_sb")
    nc.sync.dma_start(out=vscat_sb, in_=_dram_ap(tn["vscat"], 0, [[1, P], [P, NJ]]))
    xgat_sb = const.tile([P, ND], I32, tag="xgat_sb")
    nc.sync.dma_start(out=xgat_sb, in_=_dram_ap(tn["xgat"], 0, [[1, P], [P, ND]]))
    ln_gb = {}
    for nm in ("ln1_g", "ln1_b", "ln2_g", "ln2_b", "lnf_g", "lnf_b"):
        t = const.tile([P, ND], F32, tag=nm, name=nm)
        nc.sync.dma_start(out=t, in_=_dram_ap(tn[nm], 0, [[1, P], [P, ND]]))
        ln_gb[nm] = t
    # resident QKV weights, bf16 [d-part, d-tile, hk]
    wq_sb = const.tile([P, ND, H * KD], BF16, tag="wq_sb")
    wk_sb = const.tile([P, ND, H * KD], BF16, tag="wk_sb")
    wv_sb = const.tile([P, ND, H * KD], BF16, tag="wv_sb")
    for wsb, wname in ((wq_sb, "wq"), (wk_sb, "wk"), (wv_sb, "wv")):
        nc.sync.dma_start(out=wsb, in_=_dram_ap(
            tn[wname], 0, [[H * KD, P], [P * H * KD, ND], [1, H * KD]]))

    # ---------------- persistent activations ----------------
    h_sb = pers.tile([P, NJ, D], F32, tag="h_sb")
    acc = pers.tile([P, NJ, D], F32, tag="acc")
    xnT = pers.tile([P, ND, TL], BF16, tag="xnT")
    qT = pers.tile([P, NPAIR, TL], BF16, tag="qT")
    linv = pers.tile([P, NJ, H], F32, tag="linv")

    # ---------------- embedding ----------------
    for j in range(NJ):
        idx_t = small.tile([P, 1], I32, tag="idx")
        nc.sync.dma_start(out=idx_t, in_=tn["x_idx"][j * P:(j + 1) * P, :])
        ge = nc.gpsimd.indirect_dma_start(
            out=h_sb[:, j, :], out_offset=None, in_=tn["tok_emb"][:, :],
            in_offset=bass.IndirectOffsetOnAxis(ap=idx_t[:, :1], axis=0))
        if dep_from is not None:
            add_dep_helper(ge.ins, dep_from.ins, True, "repeat serialization")
        pos_t = small.tile([P, D], F32, tag="pos", bufs=1)
        nc.sync.dma_start(out=pos_t, in_=tn["pos"][j * P:(j + 1) * P, :])
        nc.vector.tensor_add(out=h_sb[:, j, :], in0=h_sb[:, j, :], in1=pos_t[:])

    # ---------------- helpers ----------------
    def layer_norm_T(g_t, b_t):
        """LN(h) over local tokens -> xnT bf16 [d-part, d-tile, t].

        Emits xnT dd-major so consumers accumulating over dd can start after
        the first dd column is complete; center/scale on GpSimd to keep DVE
        free for the stats chain."""
        mvs, rstds = [], []
        for j in range(NJ):
            stats = small.tile([P, 2, 6], F32, tag="bnst")
            mv = small.tile([P, 2], F32, tag="bnmv", bufs=4)
            for sg in range(2):
                nc.vector.bn_stats(out=stats[:, sg, :],
                                   in_=h_sb[:, j, sg * 512:(sg + 1) * 512])
            nc.vector.bn_aggr(out=mv, in_=stats)
            rstd = small.tile([P, 1], F32, tag="rstd", bufs=4)
            nc.scalar.activation(out=rstd, in_=mv[:, 1:2], func=AF.Sqrt,
                                 bias=eps_t[:, :], scale=1.0)
            nc.vector.reciprocal(out=rstd, in_=rstd)
            mvs.append(mv)
            rstds.append(rstd)
        for dd in range(ND):
            for j in range(NJ):
                xt = small.tile([P, P], BF16, tag="xt", bufs=6)
                nc.gpsimd.tensor_scalar(
                    out=xt, in0=h_sb[:, j, dd * P:(dd + 1) * P],
                    scalar1=mvs[j][:, 0:1], scalar2=rstds[j],
                    op0=ALU.subtract, op1=ALU.mult)
                tp = ps_s.tile([P, 256], BF16, tag="ps")
                nc.tensor.transpose(out=tp[:, 0:P], in_=xt[:], identity=ident_bf[:])
                nc.vector.tensor_scalar(
                    out=xnT[:, dd, j * P:(j + 1) * P], in0=tp[:, 0:P],
                    scalar1=g_t[:, dd:dd + 1], scalar2=b_t[:, dd:dd + 1],
                    op0=ALU.mult, op1=ALU.add)

    prev_reads = []

    def barrier(write_insts, read_insts_prev, name):
        if SKIP_CC:
            return None
        fin = fl_dram.tile([1, P], F32, tag="fin")
        fout = fl_dram.tile([1, P], F32, tag="fout")
        z = nc.sync.dma_start(out=fin[:, :], in_=zero_sb[:, :])
        cc = nc.gpsimd.collective_compute(
            "AllReduce", ALU.add, replica_groups=RG,
            ins=[fin[:, :]], outs=[fout[:, :]])
        add_dep_helper(cc.ins, z.ins, True, f"{name}: flag init")
        for w in write_insts:
            add_dep_helper(cc.ins, w.ins, True, f"{name}: after shared writes")
        for r in read_insts_prev:
            add_dep_helper(cc.ins, r.ins, True, f"{name}: WAR prev reads")
        return cc

    def dep_on(inst, cc, reason):
        if cc is not None:
            add_dep_helper(inst.ins, cc.ins, True, reason)

    # ---------------- transformer layers (tied weights) ----------------
    kv_pool = tc.alloc_tile_pool(name="kv_pool" + sfx, bufs=2)
    pt_pool = tc.alloc_tile_pool(name="pt_pool" + sfx, bufs=4)
    w1_pool = tc.alloc_tile_pool(name="w1_pool" + sfx, bufs=2)
    w2_pool = tc.alloc_tile_pool(name="w2_pool" + sfx, bufs=2)
    aT_pool = tc.alloc_tile_pool(name="aT_pool" + sfx, bufs=2)
    loop_pools = [kv_pool, pt_pool, w1_pool, w2_pool, aT_pool]

    for layer in range(LAYERS):
        shk = tn["sh_k0"] if layer % 2 == 0 else tn["sh_k1"]
        shv = tn["sh_v0"] if layer % 2 == 0 else tn["sh_v1"]

        layer_norm_T(ln_gb["ln1_g"], ln_gb["ln1_b"])

        # ---- K/V projections -> shared HBM scatter; Q -> SBUF ----
        k_writes = []
        kv_writes = []
        for hp in range(NPAIR):
            ps = ps_big.tile([P, TL], F32, tag="ff")
            for dd in range(ND):
                nc.tensor.matmul(
                    out=ps[:], lhsT=wk_sb[:, dd, hp * P:(hp + 1) * P],
                    rhs=xnT[:, dd, :], start=(dd == 0), stop=(dd == ND - 1))
            k_ev = ev.tile([P, TL], BF16, tag="k_ev")
            nc.scalar.activation(out=k_ev, in_=ps[:], func=AF.Identity,
                                 bias=bk_sb[:, hp:hp + 1], scale=1.0)
            w = nc.gpsimd.indirect_dma_start(
                out=shk[:, :],
                out_offset=bass.IndirectOffsetOnAxis(ap=kscat_sb[:, hp:hp + 1], axis=0),
                in_=k_ev[:], in_offset=None)
            k_writes.append(w)
        for j in range(NJ):
            v_ev = ev.tile([P, H * KD], BF16, tag="v_ev")
            for half in range(2):
                ps = ps_big.tile([P, TL], F32, tag="ff")
                for dd in range(ND):
                    nc.tensor.matmul(
                        out=ps[:], lhsT=xnT[:, dd, j * P:(j + 1) * P],
                        rhs=wv_sb[:, dd, half * 512:(half + 1) * 512],
                        start=(dd == 0), stop=(dd == ND - 1))
                nc.vector.tensor_tensor(
                    out=v_ev[:, half * 512:(half + 1) * 512], in0=ps[:],
                    in1=bv_bc[:, half * 512:(half + 1) * 512], op=ALU.add)
            w = nc.gpsimd.indirect_dma_start(
                out=shv[:, :],
                out_offset=bass.IndirectOffsetOnAxis(ap=vscat_sb[:, j:j + 1], axis=0),
                in_=v_ev[:], in_offset=None)
            kv_writes.append(w)
        cc = barrier(k_writes + kv_writes, prev_reads, f"layer{layer}")
        prev_reads = []
        # Q projection after the barrier in program order: PE fills the
        # collective wait with this work (no dep on cc).
        for hp in range(NPAIR):
            ps = ps_big.tile([P, TL], F32, tag="ff")
            for dd in range(ND):
                nc.tensor.matmul(
                    out=ps[:], lhsT=wq_sb[:, dd, hp * P:(hp + 1) * P],
                    rhs=xnT[:, dd, :], start=(dd == 0), stop=(dd == ND - 1))
            nc.scalar.activation(out=qT[:, hp, :], in_=ps[:], func=AF.Identity,
                                 bias=bq_sb[:, hp:hp + 1], scale=1.0)

        # ---- attention ----
        for hp in range(NPAIR):
            kp = kv_pool.tile([P, 2, TL], BF16, tag="kp")
            r1 = nc.sync.dma_start(out=kp, in_=_dram_ap(
                shk, hp * 2 * P * TL, [[2 * TL, P], [TL, 2], [1, TL]]))
            vp = kv_pool.tile([P, 8, 2, KD + 1], BF16, tag="vp")
            dep_on(r1, cc, "kp read after barrier")
            prev_reads.append(r1)
            for hi in range(2):
                r2 = nc.sync.dma_start(out=vp[:, :, hi, 0:KD], in_=_dram_ap(
                    shv, hp * 2 * KD + hi * KD,
                    [[H * KD, P], [P * H * KD, 8], [1, KD]]))
                dep_on(r2, cc, "vp read after barrier")
                prev_reads.append(r2)
            nc.gpsimd.memset(vp[:, :, :, KD:KD + 1], 1.0)
            for hi in range(2):
                h_ = 2 * hp + hi
                for c in range(2):
                    # k-tiles in pairs: two [128,256] score blocks share one
                    # PSUM bank -> one exp + one mask per pair
                    pairs = []
                    for half in range(2):
                        for pi in range(c + 1):
                            mi = half if pi == c else None
                            pairs.append((half, 2 * pi, mi))
                    y_ps = ps_y.tile([KD + 1, 256], F32, tag="y")
                    n_p = len(pairs)
                    for t, (half, idx0, mi) in enumerate(pairs):
                        s_ps = ps_big.tile([P, TL], F32, tag="ff")
                        for w in range(2):
                            nc.tensor.matmul(
                                out=s_ps[:, w * 256:(w + 1) * 256],
                                lhsT=kp[hi * KD:(hi + 1) * KD, half,
                                        (idx0 + w) * P:(idx0 + w + 1) * P],
                                rhs=qT[hi * KD:(hi + 1) * KD, hp,
                                       c * 256:(c + 1) * 256],
                                start=True, stop=True)
                        pt = pt_pool.tile([P, TL], BF16, tag="pt")
                        nc.scalar.activation(out=pt[:], in_=s_ps[:], func=AF.Exp,
                                             scale=SCALE)
                        if mi is not None:
                            nc.vector.tensor_tensor(
                                out=pt[:], in0=pt[:], in1=amask_sb[:, mi, :],
                                op=ALU.mult)
                        for w in range(2):
                            nc.tensor.matmul(
                                out=y_ps[:],
                                lhsT=vp[:, half * 4 + idx0 + w, hi, :],
                                rhs=pt[:, w * 256:(w + 1) * 256],
                                start=(2 * t + w == 0),
                                stop=(2 * t + w == 2 * n_p - 1))
                    y_sb = ev.tile([KD + 1, 256], BF16, tag="y_sb", bufs=3)
                    nc.vector.tensor_copy(out=y_sb[:], in_=y_ps[:])
                    for t4 in range(2):
                        o_ps = ps_s.tile([P, 256], F32, tag="ps")
                        nc.tensor.matmul(
                            out=o_ps[:, 0:KD + 1],
                            lhsT=y_sb[:, t4 * P:(t4 + 1) * P],
                            rhs=wo_sb[:, h_, :], start=True, stop=True)
                        nc.scalar.copy(
                            out=acc[:, c * 2 + t4, h_ * KD:(h_ + 1) * KD],
                            in_=o_ps[:, 0:KD])
                        nc.vector.reciprocal(
                            out=linv[:, c * 2 + t4, h_:h_ + 1],
                            in_=o_ps[:, KD:KD + 1])
        # normalize by softmax denominators + residual
        for j in range(NJ):
            lap = linv[:, j, :]
            lbc = bass.AP(tensor=lap.tensor, offset=lap.offset,
                          ap=[list(lap.ap[0]), list(lap.ap[-1]), [0, KD]])
            nc.gpsimd.tensor_tensor(
                out=acc[:, j, :].rearrange("p (h k) -> p h k", h=H),
                in0=acc[:, j, :].rearrange("p (h k) -> p h k", h=H),
                in1=lbc, op=ALU.mult)
            nc.vector.tensor_add(out=h_sb[:, j, :], in0=h_sb[:, j, :],
                                 in1=acc[:, j, :])

        layer_norm_T(ln_gb["ln2_g"], ln_gb["ln2_b"])

        # ---- FFN ----
        for grp in range(4):
            aT_g = aT_pool.tile([P, 8, TL], BF16, tag="aT")
            w1t = w1_pool.tile([P, ND, 8 * P], BF16, tag="w1")
            nc.sync.dma_start(out=w1t, in_=_dram_ap(
                tn["w1"], grp * 8 * P, [[F, P], [P * F, ND], [1, 8 * P]]))
            for fi8 in range(8):
                fi = grp * 8 + fi8
                ps = ps_big.tile([P, TL], F32, tag="ff")
                for dd in range(ND):
                    nc.tensor.matmul(
                        out=ps[:], lhsT=w1t[:, dd, fi8 * P:(fi8 + 1) * P],
                        rhs=xnT[:, dd, :], start=(dd == 0), stop=(dd == ND - 1))
                nc.scalar.activation(out=aT_g[:, fi8, :], in_=ps[:], func=AF.Relu,
                                     bias=b1_sb[:, fi:fi + 1], scale=1.0)
            w2ts = []
            for wh in range(2):
                w2t = w2_pool.tile([P, 4, D], BF16, tag="w2")
                nc.sync.dma_start(out=w2t, in_=_dram_ap(
                    tn["w2"], (grp * 8 + wh * 4) * P * D,
                    [[D, P], [P * D, 4], [1, D]]))
                w2ts.append(w2t)
            for j in range(NJ):
                for dc in range(2):
                    ffp = ps_big.tile([P, TL], F32, tag="ff")
                    for fi8 in range(8):
                        nc.tensor.matmul(
                            out=ffp[:],
                            lhsT=aT_g[:, fi8, j * P:(j + 1) * P],
                            rhs=w2ts[fi8 // 4][:, fi8 % 4, dc * 512:(dc + 1) * 512],
                            start=(fi8 == 0), stop=(fi8 == 7))
                    nc.vector.tensor_add(
                        out=h_sb[:, j, dc * 512:(dc + 1) * 512],
                        in0=h_sb[:, j, dc * 512:(dc + 1) * 512], in1=ffp[:])
        for j in range(NJ):
            nc.vector.tensor_add(out=h_sb[:, j, :], in0=h_sb[:, j, :],
                                 in1=b2_bc[:])

    for pool in reversed(loop_pools):
        pool.release()

    # ---------------- final LN + xn exchange + vocab projection ----------------
    # Own-token half runs straight from xnT with no barrier dependency, so it
    # overlaps the final collective + partner-half gather.
    layer_norm_T(ln_gb["lnf_g"], ln_gb["lnf_b"])
    x_writes = []
    for dd in range(ND):
        w = nc.gpsimd.indirect_dma_start(
            out=tn["sh_x"][:, :],
            out_offset=bass.IndirectOffsetOnAxis(ap=kscat_sb[:, dd:dd + 1], axis=0),
            in_=xnT[:, dd, :], in_offset=None)
        x_writes.append(w)
    ccf = barrier(x_writes, prev_reads, "final")

    fin_pool = tc.alloc_tile_pool(name="fin_pool" + sfx, bufs=1)
    wout_pool = tc.alloc_tile_pool(name="wout_pool" + sfx, bufs=4)
    lg_pool = tc.alloc_tile_pool(name="lg_pool" + sfx, bufs=4)
    xT_oth = fin_pool.tile([P, ND, TL], BF16, tag="xT_oth")
    for dd in range(ND):
        rr = nc.gpsimd.indirect_dma_start(
            out=xT_oth[:, dd, :],
            in_offset=bass.IndirectOffsetOnAxis(ap=xgat_sb[:, dd:dd + 1], axis=0),
            in_=tn["sh_x"][:, :], out_offset=None)
        dep_on(rr, ccf, "partner xn gather after final barrier")
    for vc2 in range(NVC // 2):
        wts = []
        for half in range(2):
            wt = wout_pool.tile([P, ND, VC], BF16, tag="wout")
            nc.sync.dma_start(out=wt, in_=_dram_ap(
                tn["wout"], (2 * vc2 + half) * VC,
                [[VSH, P], [P * VSH, ND], [1, VC]]))
            wts.append(wt)
        bout_bc = small.tile([P, 2, VC], F32, tag="bout", bufs=2)
        nc.sync.dma_start(out=bout_bc, in_=_dram_ap(
            tn["bout"], 2 * vc2 * VC, [[0, P], [VC, 2], [1, VC]]))
        for src_t, out_t in ((xnT, tn["logits_own"]), (xT_oth, tn["logits_oth"])):
            for j in range(NJ):
                lg = lg_pool.tile([P, 2, VC], BF16, tag="lg")
                for half in range(2):
                    ps = ps_big.tile([P, TL], F32, tag="ff")
                    for dd in range(ND):
                        nc.tensor.matmul(
                            out=ps[:, 0:VC],
                            lhsT=src_t[:, dd, j * P:(j + 1) * P],
                            rhs=wts[half][:, dd, :],
                            start=(dd == 0), stop=(dd == ND - 1))
                    nc.vector.tensor_tensor(out=lg[:, half, :], in0=ps[:, 0:VC],
                                            in1=bout_bc[:, half, :], op=ALU.add)
                nc.sync.dma_start(
                    out=_dram_ap(out_t, j * P * VSH + 2 * vc2 * VC,
                                 [[VSH, P], [1, 2 * VC]]),
                    in_=lg[:].rearrange("p a b -> p (a b)"))
    lg_pool.release()
    wout_pool.release()
    fin_pool.release()
    for pool in reversed(static_pools):
        pool.release()
    return ccf


_PROGRAM = None


def _get_program():
    global _PROGRAM
    if _PROGRAM is None:
        _PROGRAM = build_program()
    return _PROGRAM


def _tri():
    # tri[ki, qi] = 1 iff ki <= qi  (causal-allowed within the same tile)
    return (np.arange(P)[:, None] <= np.arange(P)[None, :]).astype(np.float32)


def make_in_maps(inputs):
    f32 = lambda k: np.ascontiguousarray(np.asarray(inputs[k], dtype=np.float32))
    bf = lambda a: np.ascontiguousarray(np.asarray(a, dtype=np.float32)).astype(NP_BF16)
    x = np.asarray(inputs["x"]).astype(np.int32)          # [B, T]
    tok_emb = f32("tok_emb")
    pos_emb = f32("pos_emb")
    wq = bf(np.asarray(inputs["Wq"]).transpose(1, 0, 2).reshape(D, H * KD))
    wk = bf(np.asarray(inputs["Wk"]).transpose(1, 0, 2).reshape(D, H * KD))
    wv = bf(np.asarray(inputs["Wv"]).transpose(1, 0, 2).reshape(D, H * KD))
    # columns 0..KD-1: per-head Wo (rows) + bo (row KD); column KD extracts the
    # softmax denominator (y row KD) through the same matmul
    wo_aug = np.zeros((KD + 1, H, KD + 1), np.float32)
    wo_aug[:KD, :, :KD] = np.asarray(inputs["Wo"], np.float32).transpose(1, 0, 2)
    wo_aug[KD, :, :KD] = np.asarray(inputs["bo"], np.float32)
    wo_aug[KD, :, KD] = 1.0
    shared = {
        "tok_emb": tok_emb,
        "wq": wq, "wk": wk, "wv": wv,
        "bq": f32("bq").reshape(H * KD), "bk": f32("bk").reshape(H * KD),
        "bv": f32("bv").reshape(1, H * KD),
        "wo_aug": bf(wo_aug),
        "w1": bf(inputs["W1"]), "b1": f32("b1"),
        "w2": bf(inputs["W2"]), "b2": f32("b2").reshape(1, D),
        "ln1_g": f32("ln1_g"), "ln1_b": f32("ln1_b"),
        "ln2_g": f32("ln2_g"), "ln2_b": f32("ln2_b"),
        "lnf_g": f32("lnf_g"), "lnf_b": f32("lnf_b"),
    }
    wout_full = bf(inputs["Wout"])
    bout_full = f32("bout").reshape(1, V)
    tri = _tri()
    ones = np.ones((P, P), np.float32)
    zeros = np.zeros((P, P), np.float32)
    masks_by_p = {
        0: [np.hstack([tri, ones, zeros, tri]),
            np.hstack([zeros, ones, zeros, zeros])],
        1: [np.hstack([ones, ones, zeros, ones]),
            np.hstack([tri, ones, zeros, tri])],
    }
    in_maps = []
    r128 = np.arange(P, dtype=np.int32)
    for c in range(8):
        b, p = c // 2, c % 2
        rows = np.concatenate(
            [np.arange((2 * j + p) * P, (2 * j + p + 1) * P) for j in range(NJ)])
        m = dict(shared)
        m["x_idx"] = np.ascontiguousarray(x[b, rows].reshape(TL, 1))
        m["pos"] = np.ascontiguousarray(pos_emb[rows])
        m["wout"] = np.ascontiguousarray(wout_full[:, p * VSH:(p + 1) * VSH])
        m["bout"] = np.ascontiguousarray(bout_full[:, p * VSH:(p + 1) * VSH])
        m["amask"] = np.stack(masks_by_p[p]).astype(NP_BF16)
        m["kscat"] = np.ascontiguousarray(
            ((np.arange(NPAIR, dtype=np.int32)[:, None] * P + r128) * 2 + p))
        m["vscat"] = np.ascontiguousarray(
            (4 * p + np.arange(NJ, dtype=np.int32)[:, None]) * P + r128)
        m["xgat"] = np.ascontiguousarray(
            ((np.arange(ND, dtype=np.int32)[:, None] * P + r128) * 2 + (1 - p)))
        in_maps.append(m)
    return in_maps


def assemble(results):
    out = np.empty((B, T, V), dtype=np.float32)
    for c in range(8):
        b, p = c // 2, c % 2
        own = np.asarray(results[c]["logits_own"], dtype=np.float32)
        oth = np.asarray(results[c]["logits_oth"], dtype=np.float32)
        for j in range(NJ):
            g_own, g_oth = 2 * j + p, 2 * j + (1 - p)
            out[b, g_own * P:(g_own + 1) * P, p * VSH:(p + 1) * VSH] = \
                own[j * P:(j + 1) * P]
            out[b, g_oth * P:(g_oth + 1) * P, p * VSH:(p + 1) * VSH] = \
                oth[j * P:(j + 1) * P]
    return out


def kernel(**inputs):
    in_maps = make_in_maps(inputs)
    nc = _get_program()
    res = run_bass_kernel_spmd(nc, in_maps, core_ids=list(range(8)))
    return assemble(res.results)


# revision 9
# speedup vs baseline: 1.0059x; 1.0059x over previous
"""Bass/Trainium2 kernel for nn_BigramLM (dense transformer, 8 NeuronCores).

Sharding v2: token-split pairs. Cores (2b, 2b+1) split batch b's tokens at
128-tile granularity: parity 0 owns global token tiles {0,2,4,6}, parity 1
owns {1,3,5,7} (512 tokens each), balancing causal attention work. All
per-token ops (LN, QKV, FFN, residual) run on local tokens only — the
transformer body is computed exactly once across the pair. Per layer the
pair exchanges K/V through pair-shared HBM (addr_space='Shared'): each core
scatter-writes its K/V half, a tiny AllReduce acts as the cross-core
barrier, then both read the full-T K/V. Attention computes each core's 512
q-tokens against a uniform k-tile schedule with per-core mask data (SPMD-
identical instruction streams). The final vocab projection splits by vocab
half after one more xn exchange. All big matmuls run in bf16 (fp32 PSUM).

Storage token order (shared buffers & logits rows): global tiles
[0,2,4,6,1,3,5,7].
"""

import os
import sys

sys.path.insert(0, "/opt/trn_rl_repo")

import numpy as np

SKIP_CC = os.environ.get("SKIP_CC", "0") == "1"   # timing A/B only: no barriers

import concourse.bass as bass
import concourse.mybir as mybir
import concourse.tile as tile
from concourse import bacc
from concourse.bass_utils import run_bass_kernel_spmd
from concourse.masks import make_identity
from concourse.tile import add_dep_helper

F32 = mybir.dt.float32
BF16 = mybir.dt.bfloat16
I32 = mybir.dt.int32
AF = mybir.ActivationFunctionType
ALU = mybir.AluOpType
NP_BF16 = mybir.dt.np(BF16)

V, D, H, KD, B, T = 32000, 1024, 16, 64, 4, 1024
F = 4 * D
LAYERS = 4
P = 128
TL = 512               # local tokens per core
NJ = 4                 # local 128-token tiles
ND = D // P            # 8
NF = F // P            # 32
NPAIR = H // 2         # 8 head pairs
VSH = V // 2           # 16000 vocab cols per core
VC = 500
NVC = VSH // VC        # 32
EPS = 1e-5
SCALE = 1.0 / float(np.sqrt(KD))
RG = [[0, 1], [2, 3], [4, 5], [6, 7]]
STORDER = [0, 2, 4, 6, 1, 3, 5, 7]   # storage tile -> global tile


def _dram_ap(handle, offset, pattern):
    t = getattr(handle, "tensor", handle)
    offset = offset + getattr(handle, "offset", 0)
    return bass.AP(tensor=t, offset=offset, ap=[list(p) for p in pattern])


def build_program(repeat=1):
    nc = bacc.Bacc("TRN2", target_bir_lowering=False, debug=False, num_devices=8)

    tn = {}
    tn["x_idx"] = nc.dram_tensor("x_idx", [TL, 1], I32, kind="ExternalInput")
    tn["tok_emb"] = nc.dram_tensor("tok_emb", [V, D], F32, kind="ExternalInput")
    tn["pos"] = nc.dram_tensor("pos", [TL, D], F32, kind="ExternalInput")
    for nm, shp, dt in (
        ("wq", [D, H * KD], BF16), ("wk", [D, H * KD], BF16),
        ("wv", [D, H * KD], BF16),
        ("bq", [H * KD], F32), ("bk", [H * KD], F32), ("bv", [1, H * KD], F32),
        ("wo_aug", [KD + 1, H, KD + 1], BF16),
        ("w1", [D, F], BF16), ("b1", [F], F32),
        ("w2", [F, D], BF16), ("b2", [1, D], F32),
        ("ln1_g", [D], F32), ("ln1_b", [D], F32),
        ("ln2_g", [D], F32), ("ln2_b", [D], F32),
        ("lnf_g", [D], F32), ("lnf_b", [D], F32),
        ("wout", [D, VSH], BF16), ("bout", [1, VSH], F32),
        ("amask", [2, P, 512], BF16),
        ("kscat", [NPAIR, P], I32), ("vscat", [NJ, P], I32),
        ("xgat", [ND, P], I32),
    ):
        tn[nm] = nc.dram_tensor(nm, shp, dt, kind="ExternalInput")
    tn["logits_own"] = nc.dram_tensor("logits_own", [TL, VSH], BF16,
                                      kind="ExternalOutput")
    tn["logits_oth"] = nc.dram_tensor("logits_oth", [TL, VSH], BF16,
                                      kind="ExternalOutput")
    # pair-shared HBM scratch (double-buffered across layers for WAR safety)
    tn["sh_k0"] = nc.dram_tensor("sh_k0", [NPAIR * P * 2, TL], BF16,
                                 kind="Internal", addr_space="Shared")
    tn["sh_k1"] = nc.dram_tensor("sh_k1", [NPAIR * P * 2, TL], BF16,
                                 kind="Internal", addr_space="Shared")
    tn["sh_v0"] = nc.dram_tensor("sh_v0", [8 * P, H * KD], BF16,
                                 kind="Internal", addr_space="Shared")
    tn["sh_v1"] = nc.dram_tensor("sh_v1", [8 * P, H * KD], BF16,
                                 kind="Internal", addr_space="Shared")
    tn["sh_x"] = nc.dram_tensor("sh_x", [ND * P * 2, TL], BF16,
                                 kind="Internal", addr_space="Shared")

    with tile.TileContext(nc) as tc:
        prev_ccf = None
        for r in range(repeat):
            prev_ccf = _body(nc, tc, tn, sfx=f"_{r}" if r else "",
                             dep_from=prev_ccf)
    nc.compile()
    return nc


def _body(nc, tc, tn, sfx="", dep_from=None):
    const = tc.alloc_tile_pool(name="const" + sfx, bufs=1)
    pers = tc.alloc_tile_pool(name="pers" + sfx, bufs=1)
    small = tc.alloc_tile_pool(name="small" + sfx, bufs=3)
    ev = tc.alloc_tile_pool(name="ev" + sfx, bufs=2)
    fl_dram = tc.alloc_tile_pool(name="fl_dram" + sfx, bufs=2, space="DRAM")
    ps_big = tc.alloc_tile_pool(name="ps_big" + sfx, bufs=3, space="PSUM")
    ps_sc = tc.alloc_tile_pool(name="ps_sc" + sfx, bufs=2, space="PSUM")
    ps_s = tc.alloc_tile_pool(name="ps_s" + sfx, bufs=2, space="PSUM")
    ps_y = tc.alloc_tile_pool(name="ps_y" + sfx, bufs=1, space="PSUM")
    static_pools = [const, pers, small, ev, fl_dram, ps_big, ps_sc, ps_s, ps_y]

    # ---------------- constants ----------------
    ident = const.tile([P, P], F32, tag="ident")
    make_identity(nc, ident)
    ident_bf = const.tile([P, P], BF16, tag="ident_bf")
    make_identity(nc, ident_bf)
    eps_t = const.tile([P, 1], F32, tag="eps_t")
    nc.vector.memset(eps_t, EPS)
    zero_sb = const.tile([1, P], F32, tag="zero_sb")
    nc.vector.memset(zero_sb, 0.0)
    ones_col = const.tile([P, 8, 2, 1], BF16, tag="ones_col")
    nc.vector.memset(ones_col, 1.0)

    bq_sb = const.tile([P, NPAIR], F32, tag="bq_sb")
    bk_sb = const.tile([P, NPAIR], F32, tag="bk_sb")
    b1_sb = const.tile([P, NF], F32, tag="b1_sb")
    nc.sync.dma_start(out=bq_sb, in_=_dram_ap(tn["bq"], 0, [[1, P], [P, NPAIR]]))
    nc.sync.dma_start(out=bk_sb, in_=_dram_ap(tn["bk"], 0, [[1, P], [P, NPAIR]]))
    nc.sync.dma_start(out=b1_sb, in_=_dram_ap(tn["b1"], 0, [[1, P], [P, NF]]))
    bv_bc = const.tile([P, H * KD], F32, tag="bv_bc")
    nc.sync.dma_start(out=bv_bc, in_=_dram_ap(tn["bv"], 0, [[0, P], [1, H * KD]]))
    b2_bc = const.tile([P, D], F32, tag="b2_bc")
    nc.sync.dma_start(out=b2_bc, in_=_dram_ap(tn["b2"], 0, [[0, P], [1, D]]))
    wo_sb = const.tile([KD + 1, H, KD + 1], BF16, tag="wo_sb")
    nc.sync.dma_start(out=wo_sb, in_=tn["wo_aug"][:, :, :])
    amask_sb = const.tile([P, 2, 512], BF16, tag="amask_sb")
    nc.sync.dma_start(out=amask_sb,
                      in_=_dram_ap(tn["amask"], 0, [[512, P], [P * 512, 2], [1, 512]]))
    kscat_sb = const.tile([P, NPAIR], I32, tag="kscat_sb")
    nc.sync.dma_start(out=kscat_sb, in_=_dram_ap(tn["kscat"], 0, [[1, P], [P, NPAIR]]))
    vscat_sb = const.tile([P, NJ], I32, tag="vscat_sb")
    nc.sync.dma_start(out=vscat_sb, in_=_dram_ap(tn["vscat"], 0, [[1, P], [P, NJ]]))
    xgat_sb = const.tile([P, ND], I32, tag="xgat_sb")
    nc.sync.dma_start(out=xgat_sb, in_=_dram_ap(tn["xgat"], 0, [[1, P], [P, ND]]))
    ln_gb = {}
    for nm in ("ln1_g", "ln1_b", "ln2_g", "ln2_b", "lnf_g", "lnf_b"):
        t = const.tile([P, ND], F32, tag=nm, name=nm)
        nc.sync.dma_start(out=t, in_=_dram_ap(tn[nm], 0, [[1, P], [P, ND]]))
        ln_gb[nm] = t
    # resident QKV weights, bf16 [d-part, d-tile, hk]
    wq_sb = const.tile([P, ND, H * KD], BF16, tag="wq_sb")
    wk_sb = const.tile([P, ND, H * KD], BF16, tag="wk_sb")
    wv_sb = const.tile([P, ND, H * KD], BF16, tag="wv_sb")
    for wsb, wname in ((wq_sb, "wq"), (wk_sb, "wk"), (wv_sb, "wv")):
        nc.sync.dma_start(out=wsb, in_=_dram_ap(
            tn[wname], 0, [[H * KD, P], [P * H * KD, ND], [1, H * KD]]))

    # ---------------- persistent activations ----------------
    h_sb = pers.tile([P, NJ, D], F32, tag="h_sb")
    acc = pers.tile([P, NJ, D], F32, tag="acc")
    xnT = pers.tile([P, ND, TL], BF16, tag="xnT")
    qT = pers.tile([P, NPAIR, TL], BF16, tag="qT")
    linv = pers.tile([P, NJ, H], F32, tag="linv")

    # ---------------- embedding ----------------
    for j in range(NJ):
        idx_t = small.tile([P, 1], I32, tag="idx")
        nc.sync.dma_start(out=idx_t, in_=tn["x_idx"][j * P:(j + 1) * P, :])
        ge = nc.gpsimd.indirect_dma_start(
            out=h_sb[:, j, :], out_offset=None, in_=tn["tok_emb"][:, :],
            in_offset=bass.IndirectOffsetOnAxis(ap=idx_t[:, :1], axis=0))
        if dep_from is not None:
            add_dep_helper(ge.ins, dep_from.ins, True, "repeat serialization")
        pos_t = small.tile([P, D], F32, tag="pos", bufs=1)
        nc.sync.dma_start(out=pos_t, in_=tn["pos"][j * P:(j + 1) * P, :])
        nc.vector.tensor_add(out=h_sb[:, j, :], in0=h_sb[:, j, :], in1=pos_t[:])

    # ---------------- helpers ----------------
    def layer_norm_T(g_t, b_t, js=tuple(range(NJ)), xt_eng=None):
        """LN(h) over local tokens -> xnT bf16 [d-part, d-tile, t].

        Emits xnT dd-major so consumers accumulating over dd can start after
        the first dd column is complete; center/scale on GpSimd to keep DVE
        free for the stats chain."""
        mvs, rstds = {}, {}
        for j in js:
            stats = small.tile([P, 2, 6], F32, tag="bnst")
            mv = small.tile([P, 2], F32, tag="bnmv", bufs=4)
            for sg in range(2):
                nc.vector.bn_stats(out=stats[:, sg, :],
                                   in_=h_sb[:, j, sg * 512:(sg + 1) * 512])
            nc.vector.bn_aggr(out=mv, in_=stats)
            rstd = small.tile([P, 1], F32, tag="rstd", bufs=4)
            nc.scalar.activation(out=rstd, in_=mv[:, 1:2], func=AF.Sqrt,
                                 bias=eps_t[:, :], scale=1.0)
            nc.vector.reciprocal(out=rstd, in_=rstd)
            mvs[j] = mv
            rstds[j] = rstd
        for dd in range(ND):
            for j in js:
                xt = small.tile([P, P], BF16, tag="xt", bufs=6)
                (xt_eng or nc.gpsimd).tensor_scalar(
                    out=xt, in0=h_sb[:, j, dd * P:(dd + 1) * P],
                    scalar1=mvs[j][:, 0:1], scalar2=rstds[j],
                    op0=ALU.subtract, op1=ALU.mult)
                tp = ps_s.tile([P, 256], BF16, tag="ps")
                nc.tensor.transpose(out=tp[:, 0:P], in_=xt[:], identity=ident_bf[:])
                nc.vector.tensor_scalar(
                    out=xnT[:, dd, j * P:(j + 1) * P], in0=tp[:, 0:P],
                    scalar1=g_t[:, dd:dd + 1], scalar2=b_t[:, dd:dd + 1],
                    op0=ALU.mult, op1=ALU.add)

    prev_reads = []

    def barrier(write_insts, read_insts_prev, name):
        if SKIP_CC:
            return None
        fin = fl_dram.tile([1, P], F32, tag="fin")
        fout = fl_dram.tile([1, P], F32, tag="fout")
        z = nc.sync.dma_start(out=fin[:, :], in_=zero_sb[:, :])
        cc = nc.gpsimd.collective_compute(
            "AllReduce", ALU.add, replica_groups=RG,
            ins=[fin[:, :]], outs=[fout[:, :]])
        add_dep_helper(cc.ins, z.ins, True, f"{name}: flag init")
        for w in write_insts:
            add_dep_helper(cc.ins, w.ins, True, f"{name}: after shared writes")
        for r in read_insts_prev:
            add_dep_helper(cc.ins, r.ins, True, f"{name}: WAR prev reads")
        return cc

    def dep_on(inst, cc, reason):
        if cc is not None:
            add_dep_helper(inst.ins, cc.ins, True, reason)

    # ---------------- transformer layers (tied weights) ----------------
    kv_pool = tc.alloc_tile_pool(name="kv_pool" + sfx, bufs=2)
    pt_pool = tc.alloc_tile_pool(name="pt_pool" + sfx, bufs=4)
    w1_pool = tc.alloc_tile_pool(name="w1_pool" + sfx, bufs=2)
    w2_pool = tc.alloc_tile_pool(name="w2_pool" + sfx, bufs=2)
    aT_pool = tc.alloc_tile_pool(name="aT_pool" + sfx, bufs=2)
    loop_pools = [kv_pool, pt_pool, w1_pool, w2_pool, aT_pool]

    for layer in range(LAYERS):
        shk = tn["sh_k0"] if layer % 2 == 0 else tn["sh_k1"]
        shv = tn["sh_v0"] if layer % 2 == 0 else tn["sh_v1"]

        layer_norm_T(ln_gb["ln1_g"], ln_gb["ln1_b"], js=(0, 1))
        layer_norm_T(ln_gb["ln1_g"], ln_gb["ln1_b"], js=(2, 3))

        # ---- K/V projections -> shared HBM scatter; Q -> SBUF ----
        k_writes = []
        kv_writes = []
        for hp in range(NPAIR):
            k_ev = ev.tile([P, TL], BF16, tag="k_ev")
            for ch in range(2):
                t0 = ch * 256
                ps = ps_sc.tile([P, 256], F32, tag="sc")
                for dd in range(ND):
                    nc.tensor.matmul(
                        out=ps[:], lhsT=wk_sb[:, dd, hp * P:(hp + 1) * P],
                        rhs=xnT[:, dd, t0:t0 + 256],
                        start=(dd == 0), stop=(dd == ND - 1))
                nc.scalar.activation(out=k_ev[:, t0:t0 + 256], in_=ps[:],
                                     func=AF.Identity,
                                     bias=bk_sb[:, hp:hp + 1], scale=1.0)
            w = nc.gpsimd.indirect_dma_start(
                out=shk[:, :],
                out_offset=bass.IndirectOffsetOnAxis(ap=kscat_sb[:, hp:hp + 1], axis=0),
                in_=k_ev[:], in_offset=None)
            k_writes.append(w)
        for j in range(NJ):
            v_ev = ev.tile([P, H * KD], BF16, tag="v_ev")
            for half in range(2):
                ps = ps_big.tile([P, TL], F32, tag="ff")
                for dd in range(ND):
                    nc.tensor.matmul(
                        out=ps[:], lhsT=xnT[:, dd, j * P:(j + 1) * P],
                        rhs=wv_sb[:, dd, half * 512:(half + 1) * 512],
                        start=(dd == 0), stop=(dd == ND - 1))
                nc.vector.tensor_tensor(
                    out=v_ev[:, half * 512:(half + 1) * 512], in0=ps[:],
                    in1=bv_bc[:, half * 512:(half + 1) * 512], op=ALU.add)
            w = nc.gpsimd.indirect_dma_start(
                out=shv[:, :],
                out_offset=bass.IndirectOffsetOnAxis(ap=vscat_sb[:, j:j + 1], axis=0),
                in_=v_ev[:], in_offset=None)
            kv_writes.append(w)
        cc = barrier(k_writes + kv_writes, prev_reads, f"layer{layer}")
        prev_reads = []
        # Q projection after the barrier in program order: PE fills the
        # collective wait with this work (no dep on cc).
        for hp in range(NPAIR):
            for ch in range(2):
                t0 = ch * 256
                ps = ps_sc.tile([P, 256], F32, tag="sc")
                for dd in range(ND):
                    nc.tensor.matmul(
                        out=ps[:], lhsT=wq_sb[:, dd, hp * P:(hp + 1) * P],
                        rhs=xnT[:, dd, t0:t0 + 256],
                        start=(dd == 0), stop=(dd == ND - 1))
                nc.scalar.activation(out=qT[:, hp, t0:t0 + 256], in_=ps[:],
                                     func=AF.Identity,
                                     bias=bq_sb[:, hp:hp + 1], scale=1.0)

        # ---- attention per 256-token q-chunk, interleaved with FFN halves ----
        def attn_chunk(c, hps=tuple(range(NPAIR))):
            for hp in hps:
                kp = kv_pool.tile([P, 2, TL], BF16, tag="kp")
                r1 = nc.sync.dma_start(out=kp, in_=_dram_ap(
                    shk, hp * 2 * P * TL, [[2 * TL, P], [TL, 2], [1, TL]]))
                vp = kv_pool.tile([P, 8, 2, KD + 1], BF16, tag="vp")
                dep_on(r1, cc, "kp read after barrier")
                prev_reads.append(r1)
                for hi in range(2):
                    r2 = nc.sync.dma_start(out=vp[:, :, hi, 0:KD], in_=_dram_ap(
                        shv, hp * 2 * KD + hi * KD,
                        [[H * KD, P], [P * H * KD, 8], [1, KD]]))
                    dep_on(r2, cc, "vp read after barrier")
                    prev_reads.append(r2)
                nc.gpsimd.memset(vp[:, :, :, KD:KD + 1], 1.0)
                for hi in range(2):
                    h_ = 2 * hp + hi
                    # k-tiles in pairs: two [128,256] score blocks share one
                    # PSUM bank -> one exp + one mask per pair
                    pairs = []
                    for half in range(2):
                        for pi in range(c + 1):
                            mi = half if pi == c else None
                            pairs.append((half, 2 * pi, mi))
                    y_ps = ps_y.tile([KD + 1, 256], F32, tag="y")
                    n_p = len(pairs)
                    for t, (half, idx0, mi) in enumerate(pairs):
                        s_ps = ps_sc.tile([P, TL], F32, tag="sc")
                        for w in range(2):
                            nc.tensor.matmul(
                                out=s_ps[:, w * 256:(w + 1) * 256],
                                lhsT=kp[hi * KD:(hi + 1) * KD, half,
                                        (idx0 + w) * P:(idx0 + w + 1) * P],
                                rhs=qT[hi * KD:(hi + 1) * KD, hp,
                                       c * 256:(c + 1) * 256],
                                start=True, stop=True)
                        pt = pt_pool.tile([P, TL], BF16, tag="pt")
                        nc.scalar.activation(out=pt[:], in_=s_ps[:], func=AF.Exp,
                                             scale=SCALE)
                        if mi is not None:
                            nc.vector.tensor_tensor(
                                out=pt[:], in0=pt[:], in1=amask_sb[:, mi, :],
                                op=ALU.mult)
                        for w in range(2):
                            nc.tensor.matmul(
                                out=y_ps[:],
                                lhsT=vp[:, half * 4 + idx0 + w, hi, :],
                                rhs=pt[:, w * 256:(w + 1) * 256],
                                start=(2 * t + w == 0),
                                stop=(2 * t + w == 2 * n_p - 1))
                    y_sb = ev.tile([KD + 1, 256], BF16, tag="y_sb", bufs=3)
                    nc.vector.tensor_copy(out=y_sb[:], in_=y_ps[:])
                    for t4 in range(2):
                        o_ps = ps_s.tile([P, 256], F32, tag="ps")
                        nc.tensor.matmul(
                            out=o_ps[:, 0:KD + 1],
                            lhsT=y_sb[:, t4 * P:(t4 + 1) * P],
                            rhs=wo_sb[:, h_, :], start=True, stop=True)
                        nc.scalar.copy(
                            out=acc[:, c * 2 + t4, h_ * KD:(h_ + 1) * KD],
                            in_=o_ps[:, 0:KD])
                        nc.vector.reciprocal(
                            out=linv[:, c * 2 + t4, h_:h_ + 1],
                            in_=o_ps[:, KD:KD + 1])

        def resid_ffn_half(ch):
            js = (2 * ch, 2 * ch + 1)
            for j in js:
                lap = linv[:, j, :]
                lbc = bass.AP(tensor=lap.tensor, offset=lap.offset,
                              ap=[list(lap.ap[0]), list(lap.ap[-1]), [0, KD]])
                nc.gpsimd.tensor_tensor(
                    out=acc[:, j, :].rearrange("p (h k) -> p h k", h=H),
                    in0=acc[:, j, :].rearrange("p (h k) -> p h k", h=H),
                    in1=lbc, op=ALU.mult)
                nc.vector.tensor_add(out=h_sb[:, j, :], in0=h_sb[:, j, :],
                                     in1=acc[:, j, :])
            layer_norm_T(ln_gb["ln2_g"], ln_gb["ln2_b"], js=js)

        def ffn_half(ch, grps=tuple(range(4))):
            js = (2 * ch, 2 * ch + 1)
            t0 = ch * 256
            for grp in grps:
                aT_g = aT_pool.tile([P, 8, 256], BF16, tag="aT")
                w1t = w1_pool.tile([P, ND, 8 * P], BF16, tag="w1")
                nc.sync.dma_start(out=w1t, in_=_dram_ap(
                    tn["w1"], grp * 8 * P, [[F, P], [P * F, ND], [1, 8 * P]]))
                for fi8 in range(8):
                    fi = grp * 8 + fi8
                    ps = ps_big.tile([P, TL], F32, tag="ff")
                    for dd in range(ND):
                        nc.tensor.matmul(
                            out=ps[:, 0:256],
                            lhsT=w1t[:, dd, fi8 * P:(fi8 + 1) * P],
                            rhs=xnT[:, dd, t0:t0 + 256],
                            start=(dd == 0), stop=(dd == ND - 1))
                    nc.scalar.activation(out=aT_g[:, fi8, :], in_=ps[:, 0:256],
                                         func=AF.Relu,
                                         bias=b1_sb[:, fi:fi + 1], scale=1.0)
                w2ts = []
                for wh in range(2):
                    w2t = w2_pool.tile([P, 4, D], BF16, tag="w2")
                    nc.sync.dma_start(out=w2t, in_=_dram_ap(
                        tn["w2"], (grp * 8 + wh * 4) * P * D,
                        [[D, P], [P * D, 4], [1, D]]))
                    w2ts.append(w2t)
                for j in js:
                    jl = j - 2 * ch
                    for dc in range(2):
                        ffp = ps_big.tile([P, TL], F32, tag="ff")
                        for fi8 in range(8):
                            nc.tensor.matmul(
                                out=ffp[:],
                                lhsT=aT_g[:, fi8, jl * P:(jl + 1) * P],
                                rhs=w2ts[fi8 // 4][:, fi8 % 4,
                                               dc * 512:(dc + 1) * 512],
                                start=(fi8 == 0), stop=(fi8 == 7))
                        nc.vector.tensor_add(
                            out=h_sb[:, j, dc * 512:(dc + 1) * 512],
                            in0=h_sb[:, j, dc * 512:(dc + 1) * 512], in1=ffp[:])
        def ffn_b2(ch):
            for j in (2 * ch, 2 * ch + 1):
                nc.vector.tensor_add(out=h_sb[:, j, :], in0=h_sb[:, j, :],
                                     in1=b2_bc[:])

        attn_chunk(0)
        resid_ffn_half(0)
        for k in range(4):
            attn_chunk(1, hps=(2 * k, 2 * k + 1))
            ffn_half(0, grps=(k,))
        ffn_b2(0)
        resid_ffn_half(1)
        ffn_half(1)
        ffn_b2(1)

    for pool in reversed(loop_pools):
        pool.release()

    # ---------------- final LN + xn exchange + vocab projection ----------------
    # Own-token half runs straight from xnT with no barrier dependency, so it
    # overlaps the final collective + partner-half gather.
    # final pools + first vocab-chunk weight prefetch BEFORE the barrier
    # machinery so the own-token vocab matmuls can fill the collective wait
    fin_pool = tc.alloc_tile_pool(name="fin_pool" + sfx, bufs=1)
    wout_pool = tc.alloc_tile_pool(name="wout_pool" + sfx, bufs=4)
    lg_pool = tc.alloc_tile_pool(name="lg_pool" + sfx, bufs=4)
    wts_pre = []
    for half in range(2):
        wt = wout_pool.tile([P, ND, VC], BF16, tag="wout")
        nc.sync.dma_start(out=wt, in_=_dram_ap(
            tn["wout"], half * VC, [[VSH, P], [P * VSH, ND], [1, VC]]))
        wts_pre.append(wt)
    bout_pre = small.tile([P, 2, VC], F32, tag="bout", bufs=2)
    nc.sync.dma_start(out=bout_pre, in_=_dram_ap(
        tn["bout"], 0, [[0, P], [VC, 2], [1, VC]]))

    layer_norm_T(ln_gb["lnf_g"], ln_gb["lnf_b"], xt_eng=nc.vector)
    x_writes = []
    for dd in range(ND):
        w = nc.gpsimd.indirect_dma_start(
            out=tn["sh_x"][:, :],
            out_offset=bass.IndirectOffsetOnAxis(ap=kscat_sb[:, dd:dd + 1], axis=0),
            in_=xnT[:, dd, :], in_offset=None)
        x_writes.append(w)
    ccf = barrier(x_writes, prev_reads, "final")

    xT_oth = fin_pool.tile([P, ND, TL], BF16, tag="xT_oth")
    for dd in range(ND):
        rr = nc.gpsimd.indirect_dma_start(
            out=xT_oth[:, dd, :],
            in_offset=bass.IndirectOffsetOnAxis(ap=xgat_sb[:, dd:dd + 1], axis=0),
            in_=tn["sh_x"][:, :], out_offset=None)
        dep_on(rr, ccf, "partner xn gather after final barrier")
    for sweep, (src_t, out_t) in enumerate(
            ((xnT, tn["logits_own"]), (xT_oth, tn["logits_oth"]))):
        for vc2 in range(NVC // 2):
            if sweep == 0 and vc2 == 0:
                wts = wts_pre
                bout_bc = bout_pre
            else:
                wts = []
                for half in range(2):
                    wt = wout_pool.tile([P, ND, VC], BF16, tag="wout")
                    nc.sync.dma_start(out=wt, in_=_dram_ap(
                        tn["wout"], (2 * vc2 + half) * VC,
                        [[VSH, P], [P * VSH, ND], [1, VC]]))
                    wts.append(wt)
                bout_bc = small.tile([P, 2, VC], F32, tag="bout", bufs=2)
                nc.sync.dma_start(out=bout_bc, in_=_dram_ap(
                    tn["bout"], 2 * vc2 * VC, [[0, P], [VC, 2], [1, VC]]))
            for j in range(NJ):
                lg = lg_pool.tile([P, 2, VC], BF16, tag="lg")
                for half in range(2):
                    ps = ps_big.tile([P, TL], F32, tag="ff")
                    for dd in range(ND):
                        nc.tensor.matmul(
                            out=ps[:, 0:VC],
                            lhsT=src_t[:, dd, j * P:(j + 1) * P],
                            rhs=wts[half][:, dd, :],
                            start=(dd == 0), stop=(dd == ND - 1))
                    nc.vector.tensor_tensor(out=lg[:, half, :], in0=ps[:, 0:VC],
                                            in1=bout_bc[:, half, :], op=ALU.add)
                nc.sync.dma_start(
                    out=_dram_ap(out_t, j * P * VSH + 2 * vc2 * VC,
                                 [[VSH, P], [1, 2 * VC]]),
                    in_=lg[:].rearrange("p a b -> p (a b)"))
    lg_pool.release()
    wout_pool.release()
    fin_pool.release()
    for pool in reversed(static_pools):
        pool.release()
    return ccf


_PROGRAM = None


def _get_program():
    global _PROGRAM
    if _PROGRAM is None:
        _PROGRAM = build_program()
    return _PROGRAM


def _tri():
    # tri[ki, qi] = 1 iff ki <= qi  (causal-allowed within the same tile)
    return (np.arange(P)[:, None] <= np.arange(P)[None, :]).astype(np.float32)


def make_in_maps(inputs):
    f32 = lambda k: np.ascontiguousarray(np.asarray(inputs[k], dtype=np.float32))
    bf = lambda a: np.ascontiguousarray(np.asarray(a, dtype=np.float32)).astype(NP_BF16)
    x = np.asarray(inputs["x"]).astype(np.int32)          # [B, T]
    tok_emb = f32("tok_emb")
    pos_emb = f32("pos_emb")
    wq = bf(np.asarray(inputs["Wq"]).transpose(1, 0, 2).reshape(D, H * KD))
    wk = bf(np.asarray(inputs["Wk"]).transpose(1, 0, 2).reshape(D, H * KD))
    wv = bf(np.asarray(inputs["Wv"]).transpose(1, 0, 2).reshape(D, H * KD))
    # columns 0..KD-1: per-head Wo (rows) + bo (row KD); column KD extracts the
    # softmax denominator (y row KD) through the same matmul
    wo_aug = np.zeros((KD + 1, H, KD + 1), np.float32)
    wo_aug[:KD, :, :KD] = np.asarray(inputs["Wo"], np.float32).transpose(1, 0, 2)
    wo_aug[KD, :, :KD] = np.asarray(inputs["bo"], np.float32)
    wo_aug[KD, :, KD] = 1.0
    shared = {
        "tok_emb": tok_emb,
        "wq": wq, "wk": wk, "wv": wv,
        "bq": f32("bq").reshape(H * KD), "bk": f32("bk").reshape(H * KD),
        "bv": f32("bv").reshape(1, H * KD),
        "wo_aug": bf(wo_aug),
        "w1": bf(inputs["W1"]), "b1": f32("b1"),
        "w2": bf(inputs["W2"]), "b2": f32("b2").reshape(1, D),
        "ln1_g": f32("ln1_g"), "ln1_b": f32("ln1_b"),
        "ln2_g": f32("ln2_g"), "ln2_b": f32("ln2_b"),
        "lnf_g": f32("lnf_g"), "lnf_b": f32("lnf_b"),
    }
    wout_full = bf(inputs["Wout"])
    bout_full = f32("bout").reshape(1, V)
    tri = _tri()
    ones = np.ones((P, P), np.float32)
    zeros = np.zeros((P, P), np.float32)
    masks_by_p = {
        0: [np.hstack([tri, ones, zeros, tri]),
            np.hstack([zeros, ones, zeros, zeros])],
        1: [np.hstack([ones, ones, zeros, ones]),
            np.hstack([tri, ones, zeros, tri])],
    }
    in_maps = []
    r128 = np.arange(P, dtype=np.int32)
    for c in range(8):
        b, p = c // 2, c % 2
        rows = np.concatenate(
            [np.arange((2 * j + p) * P, (2 * j + p + 1) * P) for j in range(NJ)])
        m = dict(shared)
        m["x_idx"] = np.ascontiguousarray(x[b, rows].reshape(TL, 1))
        m["pos"] = np.ascontiguousarray(pos_emb[rows])
        m["wout"] = np.ascontiguousarray(wout_full[:, p * VSH:(p + 1) * VSH])
        m["bout"] = np.ascontiguousarray(bout_full[:, p * VSH:(p + 1) * VSH])
        m["amask"] = np.stack(masks_by_p[p]).astype(NP_BF16)
        m["kscat"] = np.ascontiguousarray(
            ((np.arange(NPAIR, dtype=np.int32)[:, None] * P + r128) * 2 + p))
        m["vscat"] = np.ascontiguousarray(
            (4 * p + np.arange(NJ, dtype=np.int32)[:, None]) * P + r128)
        m["xgat"] = np.ascontiguousarray(
            ((np.arange(ND, dtype=np.int32)[:, None] * P + r128) * 2 + (1 - p)))
        in_maps.append(m)
    return in_maps


def assemble(results):
    out = np.empty((B, T, V), dtype=np.float32)
    for c in range(8):
        b, p = c // 2, c % 2
        own = np.asarray(results[c]["logits_own"], dtype=np.float32)
        oth = np.asarray(results[c]["logits_oth"], dtype=np.float32)
        for j in range(NJ):
            g_own, g_oth = 2 * j + p, 2 * j + (1 - p)
            out[b, g_own * P:(g_own + 1) * P, p * VSH:(p + 1) * VSH] = \
                own[j * P:(j + 1) * P]
            out[b, g_oth * P:(g_oth + 1) * P, p * VSH:(p + 1) * VSH] = \
                oth[j * P:(j + 1) * P]
    return out


def kernel(**inputs):
    in_maps = make_in_maps(inputs)
    nc = _get_program()
    res = run_bass_kernel_spmd(nc, in_maps, core_ids=list(range(8)))
    return assemble(res.results)
